# revision 30
# baseline (speedup 1.0000x reference)
"""Trainium2 Bass kernel for nn_ChannelAdaptiveNormalization.

Reference computation (per batch):
    src_n = instnorm(src); q = Wq@src_n; k = Wk@instnorm(trg); v = Wv@trg
    attn = softmax(q^T k / sqrt(C))  over t
    mean = attn @ v ; var = relu(attn @ v^2 - mean^2)
    out = sqrt(mean_s[var]) * src_n + mean_s[mean]      (broadcast over time)

Kernel decomposition (per-core, data-parallel over batch, 2 batches/core):
  * instance-norm folded into the CxC projection weights (scale columns by
    1/sd, subtract a rank-1 bias) -- normalized activations never materialize.
  * q/k/v projections in bf16; q,k,v,v^2 quantized to fp8e4 at PSUM eviction.
  * scores produced TRANSPOSED ([t, s]) as ONE DoubleRow fp8 matmul per tile
    (contraction 256 = 2x128 k-tiles); exp applies a -3.5 shift so p fits
    fp8e4 range (softmax-invariant), p stored fp8 in pair-tiles [t, 2, s].
  * Z via replicated fp8-DoubleRow ones-matmul; zinv = 1/Z on the DVE.
  * attn@v AND attn@v^2 both as fp8-DoubleRow matmuls; the old DVE a_u pass
    (64 big scalar_tensor_tensor instrs) is gone entirely:
        musum[d]  = sum_s (attn@v)[d,s]        (STT eviction x zinv, accum)
        sm2[d]    = sum_s (attn@v)[d,s]^2      (STT un*un, accum)
        av2sum[d] = sum_s (attn@v^2)[d,s]      (STT eviction x zinv, accum)
  * src kept in fp32 in SBUF for the final scale/bias (accuracy).
  * phase-interleaved emission: PE items of attn-phase-B(half h) are merged
    round-robin with scores of half h+1 (and the next batch's projections) so
    the scalar engine's exp stream (the ~74us floor) never starves.
"""

import os
import sys

import numpy as np

if "/opt/trn_rl_repo" not in sys.path:
    sys.path.insert(0, "/opt/trn_rl_repo")

from contextlib import ExitStack

import concourse.bass as bass
import concourse.tile as tile
from concourse import mybir
from concourse.bass_utils import run_bass_kernel_spmd

DT = mybir.dt
ALU = mybir.AluOpType
ACTF = mybir.ActivationFunctionType
DR = mybir.MatmulPerfMode.DoubleRow

N_CORES = 8
B_FULL = 16
B_SH = B_FULL // N_CORES  # 2 batches per core
C = 256
T = 2048
P = 128
NCH = C // P  # 2 channel chunks
NTCH = T // P  # 16 time chunks
NPAIR = NTCH // 2  # 8 DoubleRow k-tile pairs along t
EPS = 1e-5
SHIFT = 3.5  # exp(score/16 - SHIFT): keeps p inside fp8e4 range


def _build_nc() -> bass.Bass:
    nc = bass.Bass()
    src = nc.declare_dram_parameter("src", [B_SH, C, T], DT.float32, isOutput=False)
    trg = nc.declare_dram_parameter("trg", [B_SH, C, T], DT.float32, isOutput=False)
    wqt = nc.declare_dram_parameter("wqt", [C, C], DT.float32, isOutput=False)
    wkt = nc.declare_dram_parameter("wkt", [C, C], DT.float32, isOutput=False)
    wvt = nc.declare_dram_parameter("wvt", [C, C], DT.float32, isOutput=False)
    out = nc.declare_dram_parameter("out", [B_SH, C, T], DT.float32, isOutput=True)

    with tile.TileContext(nc) as tc:
        with ExitStack() as ctx:
            _build_kernel(ctx, tc, src, trg, wqt, wkt, wvt, out)
    _legalize_waits(nc)
    return nc


def _legalize_waits(nc: bass.Bass):
    """walrus on this toolchain encodes at most ONE sync wait per
    instruction (NEURON_ISA_TPB_EVENTS has a single wait slot and no
    splitting pass runs).  Hoist all but the last wait of every
    instruction into standalone single-wait EventSemaphore instructions
    on the same engine queue, which preserves ordering semantics."""
    # collect all tile-context data semaphores (skip barrier sems)
    all_sems = {}
    for fn in nc.m.functions:
        for blk in fn.blocks:
            for inst in blk.instructions:
                si = getattr(inst, "sync_info", None)
                if si is None:
                    continue
                for w in list(si.on_wait) + list(si.on_update):
                    if not w.ant_name.startswith("barrier"):
                        all_sems[w.id] = w.ant_name

    for fn in nc.m.functions:
        for blk in fn.blocks:
            snapshot = list(blk.instructions)
            for idx in range(len(snapshot) - 1, -1, -1):
                inst = snapshot[idx]
                if type(inst).__name__ == "InstISA" and getattr(inst, "isa_opcode", None) == 176:
                    # EVENT_SEMAPHORE_RANGE_CLEAR: encoding mismatches this
                    # walrus build; replace with per-sem zero-writes.
                    pos = list(blk.instructions).index(inst)
                    blk.instructions.pop(pos)
                    for sid, sname in sorted(all_sems.items()):
                        ev = mybir.InstEventSemaphore(
                            name=nc.get_next_instruction_name(), ins=[], outs=[]
                        )
                        ev.engine = inst.engine
                        ev.sync_info = mybir.SyncInfo(
                            on_wait=[],
                            on_update=[
                                mybir.SyncUpdate(
                                    sync_type="semaphore",
                                    id=sid,
                                    ant_name=sname,
                                    update_mode="sem-wr-imm",
                                    update_value=0,
                                )
                            ],
                        )
                        nc.register_instruction(ev)
                        blk.instructions.insert(pos, ev)
                        pos += 1

    for fn in nc.m.functions:
        for blk in fn.blocks:
            snapshot = list(blk.instructions)
            for idx in range(len(snapshot) - 1, -1, -1):
                inst = snapshot[idx]
                si = getattr(inst, "sync_info", None)
                if si is None or len(si.on_wait) <= 1:
                    continue
                waits = list(si.on_wait)
                evs = []
                for w in waits[:-1]:
                    ev = mybir.InstEventSemaphore(
                        name=nc.get_next_instruction_name(), ins=[], outs=[]
                    )
                    ev.engine = inst.engine
                    ev.sync_info = mybir.SyncInfo(on_wait=[w], on_update=[])
                    nc.register_instruction(ev)
                    evs.append(ev)
                si.on_wait = waits[-1:]
                inst.sync_info = si
                for ev in reversed(evs):
                    blk.instructions.insert(idx, ev)


def _interleave(*gens):
    """Round-robin drive generators to completion (order = engine queue order)."""
    live = [iter(g) for g in gens]
    while live:
        for g in list(live):
            try:
                next(g)
            except StopIteration:
                live.remove(g)


def _interleave_w(*pairs):
    """Weighted round-robin: (gen, weight) -- emit `weight` items per turn."""
    live = [[iter(g), w] for g, w in pairs]
    while live:
        for item in list(live):
            g, w = item
            for _ in range(w):
                try:
                    next(g)
                except StopIteration:
                    live.remove(item)
                    break


def _seq(*gens):
    for g in gens:
        yield from g


def _drain(gen):
    for _ in gen:
        pass


def _build_kernel(ctx, tc, src, trg, wqt, wkt, wvt, out):
    nc = tc.nc
    ep = ctx.enter_context

    sb = ep(tc.tile_pool(name="sb", bufs=1))
    ps = ep(tc.tile_pool(name="ps", bufs=1, space="PSUM"))

    # ---- constants / weights (once) ----
    ones8 = sb.tile([P, 2 * P], DT.float8e4, name="ones8", tag="ones")
    nc.vector.memset(ones8[:], 1.0)
    ones8_v = ones8[:].rearrange("p (a q) -> p a q", a=2)

    nshift = sb.tile([P, 1], DT.float32, name="nshift", tag="nshift")
    nc.vector.memset(nshift[:], -SHIFT)


    wq_bf = sb.tile([P, NCH * C], DT.bfloat16, name="wq_bf", tag="wq")
    wk_bf = sb.tile([P, NCH * C], DT.bfloat16, name="wk_bf", tag="wk")
    wv_bf = sb.tile([P, NCH * C], DT.bfloat16, name="wv_bf", tag="wv")

    def load_weight(w_bf, w_d):
        wtmp = sb.tile([P, NCH * C], DT.float32, name="wtmp", tag="wtmp", bufs=2)
        nc.gpsimd.dma_start(
            wtmp[:].rearrange("p (a d) -> p a d", a=NCH),
            w_d[:].rearrange("(a p) d -> p a d", p=P),
        )
        nc.vector.tensor_copy(w_bf[:], wtmp[:])

    load_weight(wq_bf, wqt)
    load_weight(wk_bf, wkt)

    St = [dict() for _ in range(B_SH)]
    Pp = {}  # (b, sh, pair) -> p pair tile [P, 2*1024] fp8

    # ------------------------------------------------------------------
    def emit_loads(b, casts=True):
        """DMA + fp32->bf16 casts.  src stays resident in fp32 (finals read
        it); b0's casts go on ACT/DVE (critical head), b1's on gpsimd."""
        t_f32, t_bf, s_f32, s_bf = [], [], [], []
        for cc in range(NCH):
            tf = sb.tile([P, T], DT.float32, name=f"t_f{b}_{cc}", tag=f"tf{cc}", bufs=1)
            t_f32.append(tf)
        for cc in range(NCH):
            sf = sb.tile([P, T], DT.float32, name=f"s_f{b}_{cc}", tag=f"sf{cc}", bufs=2)
            s_f32.append(sf)
        src_eng = nc.sync if b == 0 else nc.gpsimd
        for cc in range(NCH):
            src_eng.dma_start(s_f32[cc][:], src[b, cc * P : (cc + 1) * P, :])
        for h in range(2):
            for cc in range(NCH):
                nc.gpsimd.dma_start(
                    t_f32[cc][:, 1024 * h : 1024 * (h + 1)],
                    trg[b, cc * P : (cc + 1) * P, 1024 * h : 1024 * (h + 1)],
                )
            if b == 0 and h == 0:
                load_weight(wv_bf, wvt)
        St[b]["s_f32"] = s_f32
        St[b]["_f32"] = (t_f32, s_f32)
        if casts:
            emit_load_casts(b)

    def emit_load_casts(b):
        """bf16 casts into PER-HALF tiles so a consumer of half 0 never waits
        on half 1's cast (tile deps are whole-tile)."""
        emit_cast_part(b, "s")
        emit_cast_part(b, "t")

    def emit_cast_part(b, which):
        """b0: src h0 on DVE / h1 on ACT; trg h0 on DVE / h1 on gpsimd --
        emitted split around the src stats so DVE's queue serves the q-chain
        first.  b1: everything on gpsimd."""
        t_f32, s_f32 = St[b]["_f32"]
        f32s = s_f32 if which == "s" else t_f32
        store = []
        for h in range(2):
            for cc in range(NCH):
                if h == 0:
                    store.append([None, None])
                xb = sb.tile(
                    [P, 1024], DT.bfloat16,
                    name=f"{which}_bf{b}_{cc}_{h}", tag=f"{which}bf{cc}{h}", bufs=2,
                )
                store[cc][h] = xb
                sl = slice(1024 * h, 1024 * (h + 1))
                if b != 0:
                    nc.gpsimd.tensor_copy(xb[:], f32s[cc][:, sl])
                elif h == 0:
                    nc.vector.tensor_copy(xb[:], f32s[cc][:, sl])
                else:
                    nc.scalar.activation(xb[:], f32s[cc][:, sl], ACTF.Identity)
        St[b][f"{which}_bf"] = store

    # ------------------------------------------------------------------
    def rowstats(b, x_bf, nm):
        """mean + 1/sd per row.  inv = exp(-0.5*ln(var*T/(T-1))): Ln/Exp live
        in the same activation table as the attention exp."""
        bnst = sb.tile([P, 4 * 6], DT.float32, name=f"bnst_{nm}", tag="bnst", bufs=4)
        for j in range(4):
            nc.vector.bn_stats(
                bnst[:, 6 * j : 6 * (j + 1)], x_bf[j // 2][:, 512 * (j % 2) : 512 * (j % 2 + 1)]
            )
        mv = sb.tile([P, 2], DT.float32, name=f"mv_{nm}", tag=f"mv_{nm}", bufs=2)
        nc.vector.bn_aggr(mv[:], bnst[:])
        lnv = sb.tile([P, 1], DT.float32, name=f"lnv_{nm}", tag=f"lnv_{nm}", bufs=2)
        nc.scalar.activation(lnv[:], mv[:, 1:2], ACTF.Ln, scale=float(T) / (T - 1))
        inv = sb.tile([P, 1], DT.float32, name=f"inv_{nm}", tag=f"inv_{nm}", bufs=2)
        nc.scalar.activation(inv[:], lnv[:], ACTF.Exp, scale=-0.5)
        return mv[:, 0:1], inv

    def emit_stats_src(b):
        mean_s, inv_s = [], []
        for cc in range(NCH):
            m, i = rowstats(b, St[b]["s_bf"][cc], f"s{cc}")
            mean_s.append(m); inv_s.append(i)
        St[b]["mean_s"], St[b]["inv_s"] = mean_s, inv_s
        wqs = sb.tile([P, NCH * C], DT.bfloat16, name="wq_s", tag="wqs", bufs=2)
        mi_s = []
        for cc in range(NCH):
            nc.vector.tensor_scalar_mul(
                wqs[:, cc * C : (cc + 1) * C], wq_bf[:, cc * C : (cc + 1) * C], inv_s[cc][:]
            )
            mis = sb.tile([P, 1], DT.bfloat16, name=f"mi_s{cc}", tag=f"mis{cc}", bufs=2)
            nc.vector.tensor_scalar_mul(mis[:], mean_s[cc], inv_s[cc][:])
            mi_s.append(mis)
        negms = []
        for cc in range(NCH):
            ng = sb.tile([P, 1], DT.float32, name=f"negms_{cc}", tag=f"negms{cc}", bufs=2)
            nc.vector.tensor_scalar_mul(ng[:], mean_s[cc], -1.0)
            negms.append(ng)
        St[b]["wq_s"], St[b]["mi_s"], St[b]["negms"] = wqs, mi_s, negms
        for nm in ("sm", "sm2", "av2"):
            St[b][f"{nm}_slots"] = sb.tile(
                [P, 8], DT.float32, name=f"{nm}_slots{b}", tag=f"{nm}slots", bufs=2
            )

    def emit_stats_trg(b):
        t_bf = St[b]["t_bf"]
        mean_t, inv_t = [], []
        for cc in range(NCH):
            m, i = rowstats(b, t_bf[cc], f"t{cc}")
            mean_t.append(m); inv_t.append(i)
        wks = sb.tile([P, NCH * C], DT.bfloat16, name="wk_s", tag="wks", bufs=2)
        mi_t = []
        for cc in range(NCH):
            nc.vector.tensor_scalar_mul(
                wks[:, cc * C : (cc + 1) * C], wk_bf[:, cc * C : (cc + 1) * C], inv_t[cc][:]
            )
            mit = sb.tile([P, 1], DT.bfloat16, name=f"mi_t{cc}", tag=f"mit{cc}", bufs=2)
            nc.vector.tensor_scalar_mul(mit[:], mean_t[cc], inv_t[cc][:])
            mi_t.append(mit)
        St[b]["wk_s"], St[b]["mi_t"] = wks, mi_t

    def emit_stats(b):
        emit_stats_src(b)
        emit_stats_trg(b)

    # ------------------------------------------------------------------
    def emit_beta(b, w_s, mi, nm):
        bps = ps.tile([P, NCH], DT.float32, name="sps", tag="sps", bufs=2)
        for dc in range(NCH):
            for cc in range(NCH):
                nc.tensor.matmul(
                    bps[:, dc : dc + 1],
                    lhsT=w_s[:, cc * C + dc * P : cc * C + (dc + 1) * P],
                    rhs=mi[cc][:],
                    start=(cc == 0),
                    stop=(cc == NCH - 1),
                )
        nb = sb.tile([P, NCH], DT.float32, name=f"negb_{nm}", tag=f"negb{nm}", bufs=2)
        nc.vector.tensor_scalar_mul(nb[:], bps[:], -1.0)
        return nb

    def gen_proj_v(b):
        """v projection (fp8 eviction on DVE) -- no stats dependency, so it
        runs first with its evictions at the head of the DVE queue.  v^2 is
        deferred into gen_proj_kq so it doesn't delay stats/k-evicts."""
        t_bf = St[b]["t_bf"]
        v8 = sb.tile([P, NTCH * C], DT.float8e4, name="v8", tag="v8", bufs=2)
        v28 = sb.tile([P, NTCH * C], DT.float8e4, name="v28", tag="v28", bufs=2)
        St[b]["v8"], St[b]["v28"] = v8, v28
        St[b]["v8_v"] = v8[:].rearrange("p (j d) -> p j d", j=NTCH)
        St[b]["v28_v"] = v28[:].rearrange("p (j d) -> p j d", j=NTCH)
        for g in range(4):
            vps = ps.tile([P, 1024], DT.float32, name="sps", tag="sps", bufs=2)
            for j4 in range(4):
                j = 4 * g + j4
                for cc in range(NCH):
                    nc.tensor.matmul(
                        vps[:, 256 * j4 : 256 * (j4 + 1)],
                        lhsT=t_bf[cc][j // 8][:, P * (j % 8) : P * (j % 8 + 1)],
                        rhs=wv_bf[:, cc * C : (cc + 1) * C],
                        start=(cc == 0),
                        stop=(cc == NCH - 1),
                    )
            nc.vector.tensor_copy(v8[:, 1024 * g : 1024 * (g + 1)], vps[:])
            yield
        for h in range(2):
            nc.vector.tensor_mul(
                v28[:, 2048 * h : 2048 * (h + 1)],
                v8[:, 2048 * h : 2048 * (h + 1)],
                v8[:, 2048 * h : 2048 * (h + 1)],
            )

    def gen_proj_qk(b):
        """q (half 0) FIRST -- it is the longest dependency chain to the
        first scores matmul -- then k (all of t)."""
        t_bf, s_bf = St[b]["t_bf"], St[b]["s_bf"]

        kt8 = sb.tile([P, NCH * T], DT.float8e4, name="kt8", tag="kt8", bufs=2)
        qt8 = sb.tile([P, NCH * T], DT.float8e4, name="qt8", tag="qt8", bufs=2)
        St[b]["kt8"], St[b]["qt8"] = kt8, qt8
        St[b]["kt8_v"] = kt8[:].rearrange("p (a t) -> p a t", a=NCH)
        St[b]["qt8_v"] = qt8[:].rearrange("p (a t) -> p a t", a=NCH)

        negbq = emit_beta(b, St[b]["wq_s"], St[b]["mi_s"], f"q{b}")
        St[b]["negbq"] = negbq
        for dc in range(NCH):
            pps = ps.tile([P, 1024], DT.float32, name="sps", tag="sps", bufs=2)
            for cc in range(NCH):
                for n4 in range(2):
                    nc.tensor.matmul(
                        pps[:, 512 * n4 : 512 * (n4 + 1)],
                        lhsT=St[b]["wq_s"][:, cc * C + dc * P : cc * C + (dc + 1) * P],
                        rhs=s_bf[cc][0][:, 512 * n4 : 512 * (n4 + 1)],
                        start=(cc == 0),
                        stop=(cc == NCH - 1),
                    )
            if b == 0:
                nc.scalar.activation(
                    qt8[:, dc * T : dc * T + 1024], pps[:], ACTF.Identity,
                    bias=negbq[:, dc : dc + 1], scale=1.0,
                )
            else:
                nc.vector.tensor_scalar_add(
                    qt8[:, dc * T : dc * T + 1024], pps[:], negbq[:, dc : dc + 1]
                )
            yield

        negbk = emit_beta(b, St[b]["wk_s"], St[b]["mi_t"], f"k{b}")
        for half in range(2):
            for dc in range(NCH):
                pps = ps.tile([P, 1024], DT.float32, name="sps", tag="sps", bufs=2)
                for cc in range(NCH):
                    for n4 in range(2):
                        nc.tensor.matmul(
                            pps[:, 512 * n4 : 512 * (n4 + 1)],
                            lhsT=St[b]["wk_s"][:, cc * C + dc * P : cc * C + (dc + 1) * P],
                            rhs=t_bf[cc][half][:, 512 * n4 : 512 * (n4 + 1)],
                            start=(cc == 0),
                            stop=(cc == NCH - 1),
                        )
                nc.vector.tensor_scalar_add(
                    kt8[:, dc * T + 1024 * half : dc * T + 1024 * (half + 1)],
                    pps[:],
                    negbk[:, dc : dc + 1],
                )
                yield

    def gen_proj_q1(b):
        s_bf = St[b]["s_bf"]
        qt8_v = St[b]["qt8_v"]
        qt8 = St[b]["qt8"]
        for dc in range(NCH):
            pps = ps.tile([P, 1024], DT.float32, name="sps", tag="sps", bufs=2)
            for cc in range(NCH):
                for n4 in range(2):
                    nc.tensor.matmul(
                        pps[:, 512 * n4 : 512 * (n4 + 1)],
                        lhsT=St[b]["wq_s"][:, cc * C + dc * P : cc * C + (dc + 1) * P],
                        rhs=s_bf[cc][1][:, 512 * n4 : 512 * (n4 + 1)],
                        start=(cc == 0),
                        stop=(cc == NCH - 1),
                    )
            nc.vector.tensor_scalar_add(
                qt8[:, dc * T + 1024 : dc * T + 2048], pps[:],
                St[b]["negbq"][:, dc : dc + 1],
            )
            yield

    # ------------------------------------------------------------------
    def gen_scores(b, sh):
        """phase A: scores (one DoubleRow fp8 matmul per 512-chunk) + exp.
        The Z ones-matmuls ride along as each p pair completes, and zinv is
        produced at the end of the phase -- so phase B's evictions never
        stall on the softmax denominator."""
        kt8_v, qt8_v = St[b]["kt8_v"], St[b]["qt8_v"]
        so = 1024 * sh
        zz = ps.tile([P, 1024], DT.float32, name="zz", tag="zz", bufs=1)
        for tch in range(NTCH):
            j, kt = tch // 2, tch % 2
            if kt == 0:
                pp = sb.tile([P, 2048], DT.float8e4, name=f"p{b}{sh}{j}", tag="p", bufs=16)
                Pp[(b, sh, j)] = pp
            pp = Pp[(b, sh, j)]
            sps = ps.tile([P, 1024], DT.float32, name="sps", tag="sps", bufs=2)
            for n2 in range(2):
                nc.tensor.matmul(
                    sps[:, 512 * n2 : 512 * (n2 + 1)],
                    lhsT=kt8_v[:, :, P * tch : P * (tch + 1)],
                    rhs=qt8_v[:, :, so + 512 * n2 : so + 512 * (n2 + 1)],
                    perf_mode=DR,
                )
            nc.scalar.activation(
                pp[:, 1024 * kt : 1024 * (kt + 1)], sps[:], ACTF.Exp,
                scale=1.0 / 16.0, bias=nshift[:],
            )
            if kt == 1:
                pv = Pp[(b, sh, j)][:].rearrange("p (a s) -> p a s", a=2)
                for n2 in range(2):
                    nc.tensor.matmul(
                        zz[:, 512 * n2 : 512 * (n2 + 1)],
                        lhsT=ones8_v,
                        rhs=pv[:, :, 512 * n2 : 512 * (n2 + 1)],
                        start=(j == 0),
                        stop=(j == NPAIR - 1),
                        perf_mode=DR,
                        skip_group_check=True,
                    )
            yield
        # free the Z psum immediately (DVE copy) so the next half's Z
        # accumulation never waits on zln draining the scalar queue
        zzs = sb.tile([P, 1024], DT.float32, name=f"zzs{b}{sh}", tag="zzs", bufs=2)
        nc.vector.tensor_copy(zzs[:], zz[:])
        St[b][f"zz{sh}"] = zzs

    def gen_attn(b, sh, finals_per_dc=None):
        """phase B: attn@v and attn@v^2 + evictions (dc-major so finals of
        dc0 can begin while dc1 is still accumulating).  finals_per_dc maps
        dc -> generator emitted right after that dc's last eviction, so the
        slot reductions are emitted after every accumulator write they read."""
        pv = [Pp[(b, sh, j)][:].rearrange("p (a s) -> p a s", a=2) for j in range(NPAIR)]
        # zinv = exp(-ln(Z)): same ACT table as the exp stream.  Emitted here
        # (not at the end of the scores phase) so the NEXT half's first exps
        # get ahead of it on the scalar queue; the staged first chunks below
        # absorb the zinv latency on the eviction side.
        zz = St[b][f"zz{sh}"]
        zln = sb.tile([P, 1024], DT.float32, name=f"zln{b}{sh}", tag="zln", bufs=1)
        nc.scalar.activation(zln[:], zz[:], ACTF.Ln)
        zinv = sb.tile([P, 1024], DT.float32, name=f"zinv{b}{sh}", tag="zinv", bufs=2)
        nc.scalar.activation(zinv[:], zln[:], ACTF.Exp, scale=-1.0)

        first = 0
        for dc in range(NCH):
            for w_v, is_v2 in ((St[b]["v8_v"], False), (St[b]["v28_v"], True)):
                for n2 in range(2):
                    avp = ps.tile([P, 512], DT.float32, name="avp", tag="av", bufs=2)
                    for j in range(NPAIR):
                        nc.tensor.matmul(
                            avp[:],
                            lhsT=w_v[:, 2 * j : 2 * j + 2, dc * P : (dc + 1) * P],
                            rhs=pv[j][:, :, 512 * n2 : 512 * (n2 + 1)],
                            start=(j == 0),
                            stop=(j == NPAIR - 1),
                            perf_mode=DR,
                        )
                        if j == 4:
                            yield
                    src_ap = avp
                    if first < 2:
                        # stage the first chunks so their psum frees without
                        # waiting on zinv (zln/zexp still in flight on ACT)
                        stg = sb.tile([P, 512], DT.float32, name="avstg", tag="avstg", bufs=2)
                        nc.vector.tensor_copy(stg[:], avp[:])
                        src_ap = stg
                        first += 1
                    sidx = dc * 4 + sh * 2 + n2
                    if not is_v2:
                        un = sb.tile([P, 512], DT.float32, name="un", tag="un", bufs=2)
                        nc.vector.scalar_tensor_tensor(
                            out=un[:], in0=src_ap[:], scalar=1.0,
                            in1=zinv[:, 512 * n2 : 512 * (n2 + 1)],
                            op0=ALU.mult, op1=ALU.mult,
                            accum_out=St[b]["sm_slots"][:, sidx : sidx + 1],
                        )
                        jk = sb.tile([P, 512], DT.float32, name="jk", tag="jk", bufs=2)
                        nc.vector.scalar_tensor_tensor(
                            out=jk[:], in0=un[:], scalar=1.0, in1=un[:],
                            op0=ALU.mult, op1=ALU.mult,
                            accum_out=St[b]["sm2_slots"][:, sidx : sidx + 1],
                        )
                    else:
                        jk = sb.tile([P, 512], DT.float32, name="jk", tag="jk", bufs=2)
                        nc.vector.scalar_tensor_tensor(
                            out=jk[:], in0=src_ap[:], scalar=1.0,
                            in1=zinv[:, 512 * n2 : 512 * (n2 + 1)],
                            op0=ALU.mult, op1=ALU.mult,
                            accum_out=St[b]["av2_slots"][:, sidx : sidx + 1],
                        )
                    yield
            if finals_per_dc is not None:
                yield from finals_per_dc[dc]

    # ------------------------------------------------------------------
    def gen_finals_dc(b, dc):
        sm_sl, sm2_sl, av2_sl = (
            St[b]["sm_slots"], St[b]["sm2_slots"], St[b]["av2_slots"]
        )
        if True:
            sl = slice(dc * 4, (dc + 1) * 4)
            sm = sb.tile([P, 1], DT.float32, name=f"sm_{dc}", tag=f"sm{dc}", bufs=2)
            nc.vector.tensor_reduce(sm[:], sm_sl[:, sl], mybir.AxisListType.X, ALU.add)
            sm2 = sb.tile([P, 1], DT.float32, name=f"sm2_{dc}", tag=f"sm2{dc}", bufs=2)
            nc.vector.tensor_reduce(sm2[:], sm2_sl[:, sl], mybir.AxisListType.X, ALU.add)
            av2 = sb.tile([P, 1], DT.float32, name=f"av2_{dc}", tag=f"av2{dc}", bufs=2)
            nc.vector.tensor_reduce(av2[:], av2_sl[:, sl], mybir.AxisListType.X, ALU.add)
            r1 = sb.tile([P, 1], DT.float32, name=f"r1_{dc}", tag=f"r1{dc}", bufs=2)
            nc.vector.tensor_scalar(r1[:], av2[:], sm2[:], 0.0, ALU.subtract, ALU.max)
            # stdv = sqrt(r1/T) = exp(0.5*ln(r1/T)) -- no sqrt-table swap
            lnr = sb.tile([P, 1], DT.float32, name=f"lnr_{dc}", tag=f"lnr{dc}", bufs=2)
            nc.scalar.activation(lnr[:], r1[:], ACTF.Ln, scale=1.0 / T)
            stdv = sb.tile([P, 1], DT.float32, name=f"std_{dc}", tag=f"std{dc}", bufs=2)
            nc.scalar.activation(stdv[:], lnr[:], ACTF.Exp, scale=0.5)
            av = sb.tile([P, 1], DT.float32, name=f"av_{dc}", tag=f"av{dc}", bufs=2)
            nc.vector.tensor_tensor(av[:], stdv[:], St[b]["inv_s"][dc][:], ALU.mult)
            musc = sb.tile([P, 1], DT.float32, name=f"musc_{dc}", tag=f"musc{dc}", bufs=2)
            nc.vector.tensor_scalar_mul(musc[:], sm[:], 1.0 / T)
            bv = sb.tile([P, 1], DT.float32, name=f"bv_{dc}", tag=f"bv{dc}", bufs=2)
            nc.vector.scalar_tensor_tensor(
                out=bv[:], in0=av[:], scalar=St[b]["negms"][dc][:], in1=musc[:],
                op0=ALU.mult, op1=ALU.add,
            )
            for half in range(2):
                o_sb = sb.tile([P, 1024], DT.float32, name="o_sb", tag="osb", bufs=2)
                if b == 1:
                    nc.scalar.activation(
                        o_sb[:],
                        St[b]["s_f32"][dc][:, 1024 * half : 1024 * (half + 1)],
                        ACTF.Identity,
                        bias=bv[:],
                        scale=av[:],
                    )
                else:
                    nc.gpsimd.tensor_scalar(
                        o_sb[:],
                        St[b]["s_f32"][dc][:, 1024 * half : 1024 * (half + 1)],
                        av[:], bv[:], ALU.mult, ALU.add,
                    )
                nc.sync.dma_start(
                    out[b, dc * P : (dc + 1) * P, 1024 * half : 1024 * (half + 1)], o_sb[:]
                )
                yield

    def gen_finals(b):
        for dc in range(NCH):
            yield from gen_finals_dc(b, dc)

    # ------------------------------------------------------------------
    # Emission schedule: engine queues are IN-ORDER.  Interleave phase-B PE
    # work with the next half's scores so the ACT exp stream stays fed.
    emit_loads(0, casts=False)
    emit_cast_part(0, "s")
    emit_stats_src(0)
    emit_cast_part(0, "t")
    emit_stats_trg(0)
    _drain(gen_proj_qk(0))
    _interleave(gen_scores(0, 0), _seq(gen_proj_v(0), gen_proj_q1(0)))
    emit_loads(1, casts=False)  # b1 DMAs issue after b0's head is fed
    emit_load_casts(1)     # gpsimd-only: casts run during b0 attention
    emit_stats(1)
    _interleave(gen_scores(0, 1), gen_attn(0, 0))
    _interleave_w(
        (_seq(gen_proj_qk(1), gen_scores(1, 0), gen_proj_q1(1), gen_proj_v(1)), 3),
        (gen_attn(0, 1), 1),
    )
    _interleave_w((gen_scores(1, 1), 2), (gen_attn(1, 0), 1), (gen_finals(0), 1))
    _drain(gen_attn(1, 1, finals_per_dc={0: gen_finals_dc(1, 0), 1: gen_finals_dc(1, 1)}))


_NC_CACHE = None


def _get_nc():
    global _NC_CACHE
    if _NC_CACHE is None:
        _NC_CACHE = _build_nc()
    return _NC_CACHE


def _run(src, trg, Wq, Wk, Wv, **kwargs):
    src = np.ascontiguousarray(np.asarray(src, dtype=np.float32))
    trg = np.ascontiguousarray(np.asarray(trg, dtype=np.float32))
    wqt = np.ascontiguousarray(np.asarray(Wq, dtype=np.float32).T)
    wkt = np.ascontiguousarray(np.asarray(Wk, dtype=np.float32).T)
    wvt = np.ascontiguousarray(np.asarray(Wv, dtype=np.float32).T)
    nc = _get_nc()
    in_maps = [
        {
            "src": src[i * B_SH : (i + 1) * B_SH],
            "trg": trg[i * B_SH : (i + 1) * B_SH],
            "wqt": wqt,
            "wkt": wkt,
            "wvt": wvt,
        }
        for i in range(N_CORES)
    ]
    res = run_bass_kernel_spmd(nc, in_maps, list(range(N_CORES)), **kwargs)
    outp = np.concatenate([res.results[i]["out"] for i in range(N_CORES)], axis=0)
    return outp.astype(np.float32), res


def kernel(src, trg, Wq, Wk, Wv):
    outp, _ = _run(src, trg, Wq, Wk, Wv)
    return outp


# revision 31
# speedup vs baseline: 1.1062x; 1.1062x over previous
"""Trainium2 Bass kernel for nn_ChannelAdaptiveNormalization.

Reference computation (per batch):
    src_n = instnorm(src); q = Wq@src_n; k = Wk@instnorm(trg); v = Wv@trg
    attn = softmax(q^T k / sqrt(C))  over t
    mean = attn @ v ; var = relu(attn @ v^2 - mean^2)
    out = sqrt(mean_s[var]) * src_n + mean_s[mean]      (broadcast over time)

Kernel decomposition (per-core, data-parallel over batch, 2 batches/core):
  * instance-norm folded into the CxC projection weights (scale columns by
    1/sd, subtract a rank-1 bias) -- normalized activations never materialize.
  * q/k/v projections in bf16; q,k,v,v^2 quantized to fp8e4 at PSUM eviction.
  * scores produced TRANSPOSED ([t, s]) as ONE DoubleRow fp8 matmul per tile
    (contraction 256 = 2x128 k-tiles); exp applies a -3.5 shift so p fits
    fp8e4 range (softmax-invariant), p stored fp8 in pair-tiles [t, 2, s].
  * Z via replicated fp8-DoubleRow ones-matmul; zinv = 1/Z on the DVE.
  * attn@v AND attn@v^2 both as fp8-DoubleRow matmuls; the old DVE a_u pass
    (64 big scalar_tensor_tensor instrs) is gone entirely:
        musum[d]  = sum_s (attn@v)[d,s]        (STT eviction x zinv, accum)
        sm2[d]    = sum_s (attn@v)[d,s]^2      (STT un*un, accum)
        av2sum[d] = sum_s (attn@v^2)[d,s]      (STT eviction x zinv, accum)
  * src kept in fp32 in SBUF for the final scale/bias (accuracy).
  * phase-interleaved emission: PE items of attn-phase-B(half h) are merged
    round-robin with scores of half h+1 (and the next batch's projections) so
    the scalar engine's exp stream (the ~74us floor) never starves.
"""

import os
import sys

import numpy as np

if "/opt/trn_rl_repo" not in sys.path:
    sys.path.insert(0, "/opt/trn_rl_repo")

from contextlib import ExitStack

import concourse.bass as bass
import concourse.tile as tile
from concourse import mybir
from concourse.bass_utils import run_bass_kernel_spmd

DT = mybir.dt
ALU = mybir.AluOpType
ACTF = mybir.ActivationFunctionType
DR = mybir.MatmulPerfMode.DoubleRow

N_CORES = 8
B_FULL = 16
B_SH = B_FULL // N_CORES  # 2 batches per core
C = 256
T = 2048
P = 128
NCH = C // P  # 2 channel chunks
NTCH = T // P  # 16 time chunks
NPAIR = NTCH // 2  # 8 DoubleRow k-tile pairs along t
EPS = 1e-5
SHIFT = 3.5  # exp(score/16 - SHIFT): keeps p inside fp8e4 range


def _build_nc() -> bass.Bass:
    nc = bass.Bass()
    src = nc.declare_dram_parameter("src", [B_SH, C, T], DT.float32, isOutput=False)
    trg = nc.declare_dram_parameter("trg", [B_SH, C, T], DT.float32, isOutput=False)
    wqt = nc.declare_dram_parameter("wqt", [C, C], DT.float32, isOutput=False)
    wkt = nc.declare_dram_parameter("wkt", [C, C], DT.float32, isOutput=False)
    wvt = nc.declare_dram_parameter("wvt", [C, C], DT.float32, isOutput=False)
    out = nc.declare_dram_parameter("out", [B_SH, C, T], DT.float32, isOutput=True)

    with tile.TileContext(nc) as tc:
        with ExitStack() as ctx:
            _build_kernel(ctx, tc, src, trg, wqt, wkt, wvt, out)
    _legalize_waits(nc)
    return nc


def _legalize_waits(nc: bass.Bass):
    """walrus on this toolchain encodes at most ONE sync wait per
    instruction (NEURON_ISA_TPB_EVENTS has a single wait slot and no
    splitting pass runs).  Hoist all but the last wait of every
    instruction into standalone single-wait EventSemaphore instructions
    on the same engine queue, which preserves ordering semantics."""
    # collect all tile-context data semaphores (skip barrier sems)
    all_sems = {}
    for fn in nc.m.functions:
        for blk in fn.blocks:
            for inst in blk.instructions:
                si = getattr(inst, "sync_info", None)
                if si is None:
                    continue
                for w in list(si.on_wait) + list(si.on_update):
                    if not w.ant_name.startswith("barrier"):
                        all_sems[w.id] = w.ant_name

    for fn in nc.m.functions:
        for blk in fn.blocks:
            snapshot = list(blk.instructions)
            for idx in range(len(snapshot) - 1, -1, -1):
                inst = snapshot[idx]
                if type(inst).__name__ == "InstISA" and getattr(inst, "isa_opcode", None) == 176:
                    # EVENT_SEMAPHORE_RANGE_CLEAR: encoding mismatches this
                    # walrus build; replace with per-sem zero-writes.
                    pos = list(blk.instructions).index(inst)
                    blk.instructions.pop(pos)
                    for sid, sname in sorted(all_sems.items()):
                        ev = mybir.InstEventSemaphore(
                            name=nc.get_next_instruction_name(), ins=[], outs=[]
                        )
                        ev.engine = inst.engine
                        ev.sync_info = mybir.SyncInfo(
                            on_wait=[],
                            on_update=[
                                mybir.SyncUpdate(
                                    sync_type="semaphore",
                                    id=sid,
                                    ant_name=sname,
                                    update_mode="sem-wr-imm",
                                    update_value=0,
                                )
                            ],
                        )
                        nc.register_instruction(ev)
                        blk.instructions.insert(pos, ev)
                        pos += 1

    for fn in nc.m.functions:
        for blk in fn.blocks:
            snapshot = list(blk.instructions)
            for idx in range(len(snapshot) - 1, -1, -1):
                inst = snapshot[idx]
                si = getattr(inst, "sync_info", None)
                if si is None or len(si.on_wait) <= 1:
                    continue
                waits = list(si.on_wait)
                evs = []
                for w in waits[:-1]:
                    ev = mybir.InstEventSemaphore(
                        name=nc.get_next_instruction_name(), ins=[], outs=[]
                    )
                    ev.engine = inst.engine
                    ev.sync_info = mybir.SyncInfo(on_wait=[w], on_update=[])
                    nc.register_instruction(ev)
                    evs.append(ev)
                si.on_wait = waits[-1:]
                inst.sync_info = si
                for ev in reversed(evs):
                    blk.instructions.insert(idx, ev)


def _interleave(*gens):
    """Round-robin drive generators to completion (order = engine queue order)."""
    live = [iter(g) for g in gens]
    while live:
        for g in list(live):
            try:
                next(g)
            except StopIteration:
                live.remove(g)


def _interleave_w(*pairs):
    """Weighted round-robin: (gen, weight) -- emit `weight` items per turn."""
    live = [[iter(g), w] for g, w in pairs]
    while live:
        for item in list(live):
            g, w = item
            for _ in range(w):
                try:
                    next(g)
                except StopIteration:
                    live.remove(item)
                    break


def _seq(*gens):
    for g in gens:
        yield from g


def _drain(gen):
    for _ in gen:
        pass


def _build_kernel(ctx, tc, src, trg, wqt, wkt, wvt, out):
    nc = tc.nc
    ep = ctx.enter_context

    sb = ep(tc.tile_pool(name="sb", bufs=1))
    ps = ep(tc.tile_pool(name="ps", bufs=1, space="PSUM"))

    # ---- constants / weights (once) ----
    ones8 = sb.tile([P, 2 * P], DT.float8e4, name="ones8", tag="ones")
    nc.vector.memset(ones8[:], 1.0)
    ones8_v = ones8[:].rearrange("p (a q) -> p a q", a=2)

    nshift = sb.tile([P, 1], DT.float32, name="nshift", tag="nshift")
    nc.vector.memset(nshift[:], -SHIFT)


    wq_bf = sb.tile([P, NCH * C], DT.bfloat16, name="wq_bf", tag="wq")
    wk_bf = sb.tile([P, NCH * C], DT.bfloat16, name="wk_bf", tag="wk")
    wv_bf = sb.tile([P, NCH * C], DT.bfloat16, name="wv_bf", tag="wv")

    def load_weight(w_bf, w_d):
        wtmp = sb.tile([P, NCH * C], DT.float32, name="wtmp", tag="wtmp", bufs=2)
        nc.gpsimd.dma_start(
            wtmp[:].rearrange("p (a d) -> p a d", a=NCH),
            w_d[:].rearrange("(a p) d -> p a d", p=P),
        )
        nc.vector.tensor_copy(w_bf[:], wtmp[:])

    load_weight(wq_bf, wqt)
    load_weight(wk_bf, wkt)

    St = [dict() for _ in range(B_SH)]
    Pp = {}  # (b, sh, pair) -> p pair tile [P, 2*1024] fp8

    # ------------------------------------------------------------------
    def emit_loads(b, casts=True):
        """DMA + fp32->bf16 casts.  src stays resident in fp32 (finals read
        it); b0's casts go on ACT/DVE (critical head), b1's on gpsimd."""
        t_f32, t_bf, s_f32, s_bf = [], [], [], []
        for cc in range(NCH):
            tf = sb.tile([P, T], DT.float32, name=f"t_f{b}_{cc}", tag=f"tf{cc}", bufs=2)
            t_f32.append(tf)
        for cc in range(NCH):
            sf = sb.tile([P, T], DT.float32, name=f"s_f{b}_{cc}", tag=f"sf{cc}", bufs=2)
            s_f32.append(sf)
        src_eng = nc.sync if b == 0 else nc.gpsimd
        for cc in range(NCH):
            src_eng.dma_start(s_f32[cc][:], src[b, cc * P : (cc + 1) * P, :])
        for h in range(2):
            for cc in range(NCH):
                nc.gpsimd.dma_start(
                    t_f32[cc][:, 1024 * h : 1024 * (h + 1)],
                    trg[b, cc * P : (cc + 1) * P, 1024 * h : 1024 * (h + 1)],
                )
            if b == 0 and h == 0:
                load_weight(wv_bf, wvt)
        St[b]["s_f32"] = s_f32
        St[b]["_f32"] = (t_f32, s_f32)
        if casts:
            emit_load_casts(b)

    def emit_load_casts(b):
        """bf16 casts into PER-HALF tiles so a consumer of half 0 never waits
        on half 1's cast (tile deps are whole-tile)."""
        emit_cast_part(b, "s")
        emit_cast_part(b, "t")

    def emit_cast_part(b, which):
        """b0: src h0 on DVE / h1 on ACT; trg h0 on DVE / h1 on gpsimd --
        emitted split around the src stats so DVE's queue serves the q-chain
        first.  b1: everything on gpsimd."""
        t_f32, s_f32 = St[b]["_f32"]
        f32s = s_f32 if which == "s" else t_f32
        store = []
        for h in range(2):
            for cc in range(NCH):
                if h == 0:
                    store.append([None, None])
                xb = sb.tile(
                    [P, 1024], DT.bfloat16,
                    name=f"{which}_bf{b}_{cc}_{h}", tag=f"{which}bf{cc}{h}", bufs=2,
                )
                store[cc][h] = xb
                sl = slice(1024 * h, 1024 * (h + 1))
                if b != 0:
                    nc.gpsimd.tensor_copy(xb[:], f32s[cc][:, sl])
                elif h == 0:
                    nc.vector.tensor_copy(xb[:], f32s[cc][:, sl])
                else:
                    nc.scalar.activation(xb[:], f32s[cc][:, sl], ACTF.Identity)
        St[b][f"{which}_bf"] = store

    # ------------------------------------------------------------------
    def rowstats(b, x_bf, nm):
        """mean + 1/sd per row.  inv = exp(-0.5*ln(var*T/(T-1))): Ln/Exp live
        in the same activation table as the attention exp."""
        bnst = sb.tile([P, 4 * 6], DT.float32, name=f"bnst_{nm}", tag="bnst", bufs=4)
        for j in range(4):
            nc.vector.bn_stats(
                bnst[:, 6 * j : 6 * (j + 1)], x_bf[j // 2][:, 512 * (j % 2) : 512 * (j % 2 + 1)]
            )
        mv = sb.tile([P, 2], DT.float32, name=f"mv_{nm}", tag=f"mv_{nm}", bufs=2)
        nc.vector.bn_aggr(mv[:], bnst[:])
        lnv = sb.tile([P, 1], DT.float32, name=f"lnv_{nm}", tag=f"lnv_{nm}", bufs=2)
        nc.scalar.activation(lnv[:], mv[:, 1:2], ACTF.Ln, scale=float(T) / (T - 1))
        inv = sb.tile([P, 1], DT.float32, name=f"inv_{nm}", tag=f"inv_{nm}", bufs=2)
        nc.scalar.activation(inv[:], lnv[:], ACTF.Exp, scale=-0.5)
        return mv[:, 0:1], inv

    def emit_stats_src(b):
        mean_s, inv_s = [], []
        for cc in range(NCH):
            m, i = rowstats(b, St[b]["s_bf"][cc], f"s{cc}")
            mean_s.append(m); inv_s.append(i)
        St[b]["mean_s"], St[b]["inv_s"] = mean_s, inv_s
        wqs = sb.tile([P, NCH * C], DT.bfloat16, name="wq_s", tag="wqs", bufs=2)
        mi_s = []
        for cc in range(NCH):
            nc.vector.tensor_scalar_mul(
                wqs[:, cc * C : (cc + 1) * C], wq_bf[:, cc * C : (cc + 1) * C], inv_s[cc][:]
            )
            mis = sb.tile([P, 1], DT.bfloat16, name=f"mi_s{cc}", tag=f"mis{cc}", bufs=2)
            nc.vector.tensor_scalar_mul(mis[:], mean_s[cc], inv_s[cc][:])
            mi_s.append(mis)
        negms = []
        for cc in range(NCH):
            ng = sb.tile([P, 1], DT.float32, name=f"negms_{cc}", tag=f"negms{cc}", bufs=2)
            nc.vector.tensor_scalar_mul(ng[:], mean_s[cc], -1.0)
            negms.append(ng)
        St[b]["wq_s"], St[b]["mi_s"], St[b]["negms"] = wqs, mi_s, negms
        for nm in ("sm", "sm2", "av2"):
            St[b][f"{nm}_slots"] = sb.tile(
                [P, 8], DT.float32, name=f"{nm}_slots{b}", tag=f"{nm}slots", bufs=2
            )

    def emit_stats_trg(b):
        t_bf = St[b]["t_bf"]
        mean_t, inv_t = [], []
        for cc in range(NCH):
            m, i = rowstats(b, t_bf[cc], f"t{cc}")
            mean_t.append(m); inv_t.append(i)
        wks = sb.tile([P, NCH * C], DT.bfloat16, name="wk_s", tag="wks", bufs=2)
        mi_t = []
        for cc in range(NCH):
            nc.vector.tensor_scalar_mul(
                wks[:, cc * C : (cc + 1) * C], wk_bf[:, cc * C : (cc + 1) * C], inv_t[cc][:]
            )
            mit = sb.tile([P, 1], DT.bfloat16, name=f"mi_t{cc}", tag=f"mit{cc}", bufs=2)
            nc.vector.tensor_scalar_mul(mit[:], mean_t[cc], inv_t[cc][:])
            mi_t.append(mit)
        St[b]["wk_s"], St[b]["mi_t"] = wks, mi_t

    def emit_stats(b):
        emit_stats_src(b)
        emit_stats_trg(b)

    # ------------------------------------------------------------------
    def emit_beta(b, w_s, mi, nm):
        bps = ps.tile([P, NCH], DT.float32, name="sps", tag="sps", bufs=2)
        for dc in range(NCH):
            for cc in range(NCH):
                nc.tensor.matmul(
                    bps[:, dc : dc + 1],
                    lhsT=w_s[:, cc * C + dc * P : cc * C + (dc + 1) * P],
                    rhs=mi[cc][:],
                    start=(cc == 0),
                    stop=(cc == NCH - 1),
                )
        nb = sb.tile([P, NCH], DT.float32, name=f"negb_{nm}", tag=f"negb{nm}", bufs=2)
        nc.vector.tensor_scalar_mul(nb[:], bps[:], -1.0)
        return nb

    def gen_proj_v(b):
        """v projection (fp8 eviction on DVE) -- no stats dependency, so it
        runs first with its evictions at the head of the DVE queue.  v^2 is
        deferred into gen_proj_kq so it doesn't delay stats/k-evicts."""
        t_bf = St[b]["t_bf"]
        v8 = sb.tile([P, NTCH * C], DT.float8e4, name="v8", tag="v8", bufs=2)
        v28 = sb.tile([P, NTCH * C], DT.float8e4, name="v28", tag="v28", bufs=2)
        St[b]["v8"], St[b]["v28"] = v8, v28
        St[b]["v8_v"] = v8[:].rearrange("p (j d) -> p j d", j=NTCH)
        St[b]["v28_v"] = v28[:].rearrange("p (j d) -> p j d", j=NTCH)
        for g in range(4):
            vps = ps.tile([P, 1024], DT.float32, name="sps", tag="sps", bufs=2)
            for j4 in range(4):
                j = 4 * g + j4
                for cc in range(NCH):
                    nc.tensor.matmul(
                        vps[:, 256 * j4 : 256 * (j4 + 1)],
                        lhsT=t_bf[cc][j // 8][:, P * (j % 8) : P * (j % 8 + 1)],
                        rhs=wv_bf[:, cc * C : (cc + 1) * C],
                        start=(cc == 0),
                        stop=(cc == NCH - 1),
                    )
            nc.vector.tensor_copy(v8[:, 1024 * g : 1024 * (g + 1)], vps[:])
            yield
        for h in range(2):
            nc.vector.tensor_mul(
                v28[:, 2048 * h : 2048 * (h + 1)],
                v8[:, 2048 * h : 2048 * (h + 1)],
                v8[:, 2048 * h : 2048 * (h + 1)],
            )

    def gen_proj_qk(b):
        """q (half 0) FIRST -- it is the longest dependency chain to the
        first scores matmul -- then k (all of t)."""
        t_bf, s_bf = St[b]["t_bf"], St[b]["s_bf"]

        kt8 = sb.tile([P, NCH * T], DT.float8e4, name="kt8", tag="kt8", bufs=2)
        qt8 = sb.tile([P, NCH * T], DT.float8e4, name="qt8", tag="qt8", bufs=2)
        St[b]["kt8"], St[b]["qt8"] = kt8, qt8
        St[b]["kt8_v"] = kt8[:].rearrange("p (a t) -> p a t", a=NCH)
        St[b]["qt8_v"] = qt8[:].rearrange("p (a t) -> p a t", a=NCH)

        negbq = emit_beta(b, St[b]["wq_s"], St[b]["mi_s"], f"q{b}")
        St[b]["negbq"] = negbq
        for dc in range(NCH):
            pps = ps.tile([P, 1024], DT.float32, name="sps", tag="sps", bufs=2)
            for cc in range(NCH):
                for n4 in range(2):
                    nc.tensor.matmul(
                        pps[:, 512 * n4 : 512 * (n4 + 1)],
                        lhsT=St[b]["wq_s"][:, cc * C + dc * P : cc * C + (dc + 1) * P],
                        rhs=s_bf[cc][0][:, 512 * n4 : 512 * (n4 + 1)],
                        start=(cc == 0),
                        stop=(cc == NCH - 1),
                    )
            if b == 0:
                nc.scalar.activation(
                    qt8[:, dc * T : dc * T + 1024], pps[:], ACTF.Identity,
                    bias=negbq[:, dc : dc + 1], scale=1.0,
                )
            else:
                nc.vector.tensor_scalar_add(
                    qt8[:, dc * T : dc * T + 1024], pps[:], negbq[:, dc : dc + 1]
                )
            yield

        negbk = emit_beta(b, St[b]["wk_s"], St[b]["mi_t"], f"k{b}")
        for half in range(2):
            for dc in range(NCH):
                pps = ps.tile([P, 1024], DT.float32, name="sps", tag="sps", bufs=2)
                for cc in range(NCH):
                    for n4 in range(2):
                        nc.tensor.matmul(
                            pps[:, 512 * n4 : 512 * (n4 + 1)],
                            lhsT=St[b]["wk_s"][:, cc * C + dc * P : cc * C + (dc + 1) * P],
                            rhs=t_bf[cc][half][:, 512 * n4 : 512 * (n4 + 1)],
                            start=(cc == 0),
                            stop=(cc == NCH - 1),
                        )
                nc.vector.tensor_scalar_add(
                    kt8[:, dc * T + 1024 * half : dc * T + 1024 * (half + 1)],
                    pps[:],
                    negbk[:, dc : dc + 1],
                )
                yield

    def gen_proj_q1(b):
        s_bf = St[b]["s_bf"]
        qt8_v = St[b]["qt8_v"]
        qt8 = St[b]["qt8"]
        for dc in range(NCH):
            pps = ps.tile([P, 1024], DT.float32, name="sps", tag="sps", bufs=2)
            for cc in range(NCH):
                for n4 in range(2):
                    nc.tensor.matmul(
                        pps[:, 512 * n4 : 512 * (n4 + 1)],
                        lhsT=St[b]["wq_s"][:, cc * C + dc * P : cc * C + (dc + 1) * P],
                        rhs=s_bf[cc][1][:, 512 * n4 : 512 * (n4 + 1)],
                        start=(cc == 0),
                        stop=(cc == NCH - 1),
                    )
            nc.vector.tensor_scalar_add(
                qt8[:, dc * T + 1024 : dc * T + 2048], pps[:],
                St[b]["negbq"][:, dc : dc + 1],
            )
            yield

    # ------------------------------------------------------------------
    def gen_scores(b, sh):
        """phase A: scores (one DoubleRow fp8 matmul per 512-chunk) + exp.
        The Z ones-matmuls ride along as each p pair completes, and zinv is
        produced at the end of the phase -- so phase B's evictions never
        stall on the softmax denominator."""
        kt8_v, qt8_v = St[b]["kt8_v"], St[b]["qt8_v"]
        so = 1024 * sh
        zz = ps.tile([P, 1024], DT.float32, name="zz", tag="zz", bufs=1)
        for tch in range(NTCH):
            j, kt = tch // 2, tch % 2
            if kt == 0:
                pp = sb.tile([P, 2048], DT.float8e4, name=f"p{b}{sh}{j}", tag="p", bufs=16)
                Pp[(b, sh, j)] = pp
            pp = Pp[(b, sh, j)]
            sps = ps.tile([P, 1024], DT.float32, name="sps", tag="sps", bufs=2)
            for n2 in range(2):
                nc.tensor.matmul(
                    sps[:, 512 * n2 : 512 * (n2 + 1)],
                    lhsT=kt8_v[:, :, P * tch : P * (tch + 1)],
                    rhs=qt8_v[:, :, so + 512 * n2 : so + 512 * (n2 + 1)],
                    perf_mode=DR,
                )
            nc.scalar.activation(
                pp[:, 1024 * kt : 1024 * (kt + 1)], sps[:], ACTF.Exp,
                scale=1.0 / 16.0, bias=nshift[:],
            )
            if kt == 1:
                pv = Pp[(b, sh, j)][:].rearrange("p (a s) -> p a s", a=2)
                for n2 in range(2):
                    nc.tensor.matmul(
                        zz[:, 512 * n2 : 512 * (n2 + 1)],
                        lhsT=ones8_v,
                        rhs=pv[:, :, 512 * n2 : 512 * (n2 + 1)],
                        start=(j == 0),
                        stop=(j == NPAIR - 1),
                        perf_mode=DR,
                        skip_group_check=True,
                    )
            yield
        St[b][f"zz{sh}"] = zz

    def gen_attn(b, sh, finals_per_dc=None):
        """phase B: attn@v and attn@v^2 + evictions (dc-major so finals of
        dc0 can begin while dc1 is still accumulating).  finals_per_dc maps
        dc -> generator emitted right after that dc's last eviction, so the
        slot reductions are emitted after every accumulator write they read."""
        pv = [Pp[(b, sh, j)][:].rearrange("p (a s) -> p a s", a=2) for j in range(NPAIR)]
        # zinv = exp(-ln(Z)): same ACT table as the exp stream.  Emitted here
        # (not at the end of the scores phase) so the NEXT half's first exps
        # get ahead of it on the scalar queue; the staged first chunks below
        # absorb the zinv latency on the eviction side.
        zz = St[b][f"zz{sh}"]
        zln = sb.tile([P, 1024], DT.float32, name=f"zln{b}{sh}", tag="zln", bufs=1)
        nc.scalar.activation(zln[:], zz[:], ACTF.Ln)
        zinv = sb.tile([P, 1024], DT.float32, name=f"zinv{b}{sh}", tag="zinv", bufs=2)
        nc.scalar.activation(zinv[:], zln[:], ACTF.Exp, scale=-1.0)

        first = 0
        for dc in range(NCH):
            for w_v, is_v2 in ((St[b]["v8_v"], False), (St[b]["v28_v"], True)):
                for n2 in range(2):
                    avp = ps.tile([P, 512], DT.float32, name="avp", tag="av", bufs=2)
                    for j in range(NPAIR):
                        nc.tensor.matmul(
                            avp[:],
                            lhsT=w_v[:, 2 * j : 2 * j + 2, dc * P : (dc + 1) * P],
                            rhs=pv[j][:, :, 512 * n2 : 512 * (n2 + 1)],
                            start=(j == 0),
                            stop=(j == NPAIR - 1),
                            perf_mode=DR,
                        )
                        if j == 4:
                            yield
                    src_ap = avp
                    if first < 2:
                        # stage the first chunks so their psum frees without
                        # waiting on zinv (zln/zexp still in flight on ACT)
                        stg = sb.tile([P, 512], DT.float32, name="avstg", tag="avstg", bufs=2)
                        nc.vector.tensor_copy(stg[:], avp[:])
                        src_ap = stg
                        first += 1
                    sidx = dc * 4 + sh * 2 + n2
                    if not is_v2:
                        un = sb.tile([P, 512], DT.float32, name="un", tag="un", bufs=2)
                        nc.vector.scalar_tensor_tensor(
                            out=un[:], in0=src_ap[:], scalar=1.0,
                            in1=zinv[:, 512 * n2 : 512 * (n2 + 1)],
                            op0=ALU.mult, op1=ALU.mult,
                            accum_out=St[b]["sm_slots"][:, sidx : sidx + 1],
                        )
                        jk = sb.tile([P, 512], DT.float32, name="jk", tag="jk", bufs=2)
                        nc.vector.scalar_tensor_tensor(
                            out=jk[:], in0=un[:], scalar=1.0, in1=un[:],
                            op0=ALU.mult, op1=ALU.mult,
                            accum_out=St[b]["sm2_slots"][:, sidx : sidx + 1],
                        )
                    else:
                        jk = sb.tile([P, 512], DT.float32, name="jk", tag="jk", bufs=2)
                        nc.vector.scalar_tensor_tensor(
                            out=jk[:], in0=src_ap[:], scalar=1.0,
                            in1=zinv[:, 512 * n2 : 512 * (n2 + 1)],
                            op0=ALU.mult, op1=ALU.mult,
                            accum_out=St[b]["av2_slots"][:, sidx : sidx + 1],
                        )
                    yield
            if finals_per_dc is not None:
                yield from finals_per_dc[dc]

    # ------------------------------------------------------------------
    def gen_finals_dc(b, dc):
        sm_sl, sm2_sl, av2_sl = (
            St[b]["sm_slots"], St[b]["sm2_slots"], St[b]["av2_slots"]
        )
        if True:
            sl = slice(dc * 4, (dc + 1) * 4)
            sm = sb.tile([P, 1], DT.float32, name=f"sm_{dc}", tag=f"sm{dc}", bufs=2)
            nc.vector.tensor_reduce(sm[:], sm_sl[:, sl], mybir.AxisListType.X, ALU.add)
            sm2 = sb.tile([P, 1], DT.float32, name=f"sm2_{dc}", tag=f"sm2{dc}", bufs=2)
            nc.vector.tensor_reduce(sm2[:], sm2_sl[:, sl], mybir.AxisListType.X, ALU.add)
            av2 = sb.tile([P, 1], DT.float32, name=f"av2_{dc}", tag=f"av2{dc}", bufs=2)
            nc.vector.tensor_reduce(av2[:], av2_sl[:, sl], mybir.AxisListType.X, ALU.add)
            r1 = sb.tile([P, 1], DT.float32, name=f"r1_{dc}", tag=f"r1{dc}", bufs=2)
            nc.vector.tensor_scalar(r1[:], av2[:], sm2[:], 0.0, ALU.subtract, ALU.max)
            # stdv = sqrt(r1/T) = exp(0.5*ln(r1/T)) -- no sqrt-table swap
            lnr = sb.tile([P, 1], DT.float32, name=f"lnr_{dc}", tag=f"lnr{dc}", bufs=2)
            nc.scalar.activation(lnr[:], r1[:], ACTF.Ln, scale=1.0 / T)
            stdv = sb.tile([P, 1], DT.float32, name=f"std_{dc}", tag=f"std{dc}", bufs=2)
            nc.scalar.activation(stdv[:], lnr[:], ACTF.Exp, scale=0.5)
            av = sb.tile([P, 1], DT.float32, name=f"av_{dc}", tag=f"av{dc}", bufs=2)
            nc.vector.tensor_tensor(av[:], stdv[:], St[b]["inv_s"][dc][:], ALU.mult)
            musc = sb.tile([P, 1], DT.float32, name=f"musc_{dc}", tag=f"musc{dc}", bufs=2)
            nc.vector.tensor_scalar_mul(musc[:], sm[:], 1.0 / T)
            bv = sb.tile([P, 1], DT.float32, name=f"bv_{dc}", tag=f"bv{dc}", bufs=2)
            nc.vector.scalar_tensor_tensor(
                out=bv[:], in0=av[:], scalar=St[b]["negms"][dc][:], in1=musc[:],
                op0=ALU.mult, op1=ALU.add,
            )
            for half in range(2):
                o_sb = sb.tile([P, 1024], DT.float32, name="o_sb", tag="osb", bufs=2)
                if b == 1:
                    nc.scalar.activation(
                        o_sb[:],
                        St[b]["s_f32"][dc][:, 1024 * half : 1024 * (half + 1)],
                        ACTF.Identity,
                        bias=bv[:],
                        scale=av[:],
                    )
                else:
                    nc.gpsimd.tensor_scalar(
                        o_sb[:],
                        St[b]["s_f32"][dc][:, 1024 * half : 1024 * (half + 1)],
                        av[:], bv[:], ALU.mult, ALU.add,
                    )
                nc.sync.dma_start(
                    out[b, dc * P : (dc + 1) * P, 1024 * half : 1024 * (half + 1)], o_sb[:]
                )
                yield

    def gen_finals(b):
        for dc in range(NCH):
            yield from gen_finals_dc(b, dc)

    # ------------------------------------------------------------------
    # Emission schedule: engine queues are IN-ORDER.  Interleave phase-B PE
    # work with the next half's scores so the ACT exp stream stays fed.
    emit_loads(0, casts=False)
    emit_cast_part(0, "s")
    emit_stats_src(0)
    emit_cast_part(0, "t")
    emit_stats_trg(0)
    _drain(gen_proj_qk(0))
    _interleave(gen_scores(0, 0), _seq(gen_proj_v(0), gen_proj_q1(0)))
    emit_loads(1, casts=False)  # b1 DMAs issue after b0's head is fed
    emit_load_casts(1)     # gpsimd-only: casts run during b0 attention
    emit_stats(1)
    _interleave(gen_scores(0, 1), gen_attn(0, 0))
    _interleave_w(
        (_seq(gen_proj_qk(1), gen_scores(1, 0), gen_proj_q1(1), gen_proj_v(1)), 3),
        (gen_attn(0, 1), 1),
    )
    _interleave_w((gen_scores(1, 1), 2), (gen_attn(1, 0), 1), (gen_finals(0), 1))
    _drain(gen_attn(1, 1, finals_per_dc={0: gen_finals_dc(1, 0), 1: gen_finals_dc(1, 1)}))


_NC_CACHE = None


def _get_nc():
    global _NC_CACHE
    if _NC_CACHE is None:
        _NC_CACHE = _build_nc()
    return _NC_CACHE


def _run(src, trg, Wq, Wk, Wv, **kwargs):
    src = np.ascontiguousarray(np.asarray(src, dtype=np.float32))
    trg = np.ascontiguousarray(np.asarray(trg, dtype=np.float32))
    wqt = np.ascontiguousarray(np.asarray(Wq, dtype=np.float32).T)
    wkt = np.ascontiguousarray(np.asarray(Wk, dtype=np.float32).T)
    wvt = np.ascontiguousarray(np.asarray(Wv, dtype=np.float32).T)
    nc = _get_nc()
    in_maps = [
        {
            "src": src[i * B_SH : (i + 1) * B_SH],
            "trg": trg[i * B_SH : (i + 1) * B_SH],
            "wqt": wqt,
            "wkt": wkt,
            "wvt": wvt,
        }
        for i in range(N_CORES)
    ]
    res = run_bass_kernel_spmd(nc, in_maps, list(range(N_CORES)), **kwargs)
    outp = np.concatenate([res.results[i]["out"] for i in range(N_CORES)], axis=0)
    return outp.astype(np.float32), res


def kernel(src, trg, Wq, Wk, Wv):
    outp, _ = _run(src, trg, Wq, Wk, Wv)
    return outp
